# revision 1
# baseline (speedup 1.0000x reference)
"""ChannelAttention TRN2 Bass kernel.

Math (per sample):
  xf = x.reshape(C, L)
  G  = xf @ xf.T                      [C, C]   (Gram over spatial dim)
  S  = Wq @ G @ Wk.T                  [C, C]   (== Q @ K.T)
  A  = softmax(S, axis=1)
  M  = A @ Wv                         [C, C]
  y  = gama * (M @ xf) + xf           [C, L]

Sharding: data-parallel over batch (16 samples / 8 cores = 2 per core).

Precision: Gram + value path in fp16 (PE runs fp16 at 1 cyc/row, fp32 at
4); S-chain in hi/lo-split fp16 (3 passes ~= 22 mantissa bits); softmax
and the +x residual in fp32.

Schedule (phases of the two samples interleaved; OUT0 and TG1 are
chunk-interleaved so TG1's PE work covers OUT0's DMA-bound window):
  TG0 W S0 AT0 MT0 [OUT0 x TG1] S1 AT1 MT1 OUT1
"""
import numpy as np

N_CORES = 8
N, C, H, W = 16, 512, 64, 64
L = H * W            # 4096
NS = N // N_CORES    # samples per core
P = 128              # partitions
KT = C // P          # 4 c-tiles
LT = L // P          # 32 l-tiles
NCH2 = L // 1024     # 4 super-chunks of 1024 columns


def _build(trace_scopes=False, repeat=None):
    import os
    import concourse.bass as bass
    import concourse.mybir as mybir
    import concourse.tile as tile
    from concourse import bacc
    from concourse.masks import make_identity
    from concourse.bass import ds

    f32 = mybir.dt.float32
    f16 = mybir.dt.float16
    AF = mybir.ActivationFunctionType

    if repeat is None:
        repeat = int(os.environ.get("KERNEL_BUILD_REPEAT", "1"))

    nc = bacc.Bacc("TRN2", debug=False)
    x_d = nc.dram_tensor("x", [NS, C, L], f32, kind="ExternalInput")
    wq_d = nc.dram_tensor("Wq", [C, C], f32, kind="ExternalInput")
    wk_d = nc.dram_tensor("Wk", [C, C], f32, kind="ExternalInput")
    wv_d = nc.dram_tensor("Wv", [C, C], f32, kind="ExternalInput")
    gama_d = nc.dram_tensor("gama", [1], f32, kind="ExternalInput")
    i16_d = nc.dram_tensor("ident16_in", [P, P], f16, kind="ExternalInput")
    i32_d = nc.dram_tensor("ident32_in", [P, P], f32, kind="ExternalInput")
    y_d = nc.dram_tensor("y", [NS, C, L], f32, kind="ExternalOutput")

    with tile.TileContext(nc) as tc:
        from contextlib import ExitStack
        ctx = ExitStack()
        with ctx:
            consts = ctx.enter_context(tc.tile_pool(name="consts", bufs=1))
            wpool = ctx.enter_context(tc.tile_pool(name="wpool", bufs=1))
            ypool = ctx.enter_context(tc.tile_pool(name="ypool", bufs=1))
            sbuf = ctx.enter_context(tc.tile_pool(name="sbuf", bufs=1))
            gpool = ctx.enter_context(tc.tile_pool(name="gpool", bufs=1))
            stream = ctx.enter_context(tc.tile_pool(name="stream", bufs=3))
            stage = ctx.enter_context(tc.tile_pool(name="stage", bufs=2))
            stats = ctx.enter_context(tc.tile_pool(name="stats", bufs=4))
            ps = ctx.enter_context(tc.tile_pool(name="ps", bufs=1, space="PSUM"))

            ident16 = consts.tile([P, P], f16)
            nc.sync.dma_start(out=ident16[:], in_=i16_d.ap())
            ident = consts.tile([P, P], f32)
            nc.sync.dma_start(out=ident[:], in_=i32_d.ap())
            gama_sb = consts.tile([P, 1], f32)
            nc.gpsimd.dma_start(out=gama_sb[:], in_=gama_d.ap().to_broadcast((P, 1)))

            # weight tiles (filled by phase_W)
            wv16_sb = wpool.tile([P, KT, C], f16)  # Wv natural: [k part, k-tile, c]
            wqTh = wpool.tile([P, KT, C], f16)     # Wq^T hi: [c part, c-tile, q]
            wqTl = wpool.tile([P, KT, C], f16)
            wkTh = wpool.tile([P, KT, C], f16)
            wkTl = wpool.tile([P, KT, C], f16)

            def phase_W():
                wvn = stream.tile([P, KT, C], f32, tag="stream", name="wvn")
                nc.sync.dma_start(
                    out=wvn[:], in_=wv_d.ap().rearrange("(t p) c -> p t c", p=P))
                nc.vector.tensor_copy(out=wv16_sb[:], in_=wvn[:])
                for w_d, wTh, wTl in ((wq_d, wqTh, wqTl), (wk_d, wkTh, wkTl)):
                    wn = stream.tile([P, KT, C], f32, tag="stream", name="wn")
                    nc.sync.dma_start(
                        out=wn[:], in_=w_d.ap().rearrange("(t p) c -> p t c", p=P))
                    for ct in range(KT):
                        ptw = ps.tile([P, C], f32, tag="w2", bufs=2, name="ptw")
                        for qt in range(KT):
                            nc.tensor.transpose(
                                ptw[:, ds(qt * P, P)], wn[:, qt, ds(ct * P, P)],
                                ident[:])
                        nc.scalar.copy(out=wTh[:, ct, :], in_=ptw[:])
                        nc.vector.tensor_sub(wTl[:, ct, :], ptw[:], wTh[:, ct, :])

            # per-sample state kept across interleaved phases
            ghs = [None] * NS
            gls = [None] * NS
            a16s = [None] * NS
            ats = [None] * NS
            mts = [None] * NS
            pre = {}

            def x_re(s):
                return x_d.ap()[s].rearrange("(t p) l -> p t l", p=P)  # [128,KT,L]

            def tg_phase(s):
                """Returns (chunk_fns, finalize_fn): stream x 2MB super-chunks,
                cast fp16, transpose on PE, accumulate Gram per chunk."""
                x_s = x_re(s)
                y_sb = ypool.tile([P, LT, C], f16, tag="Y", name=f"y{s}")
                state = {}

                def chunk(lc):
                    if lc == 0:
                        state["g_ps"] = [
                            ps.tile([P, C], f32, tag="acc", bufs=4,
                                    name=f"g{s}_{m}") for m in range(KT)]
                    g_ps = state["g_ps"]
                    xs32 = stream.tile([P, KT, 1024], f32, tag="stream",
                                       name="xs32")
                    if lc == 0:
                        for q in range(4):
                            nc.sync.dma_start(
                                out=xs32[:, :, ds(q * 256, 256)],
                                in_=x_s[:, :, ds(q * 256, 256)])
                    else:
                        nc.sync.dma_start(out=xs32[:],
                                          in_=x_s[:, :, ds(lc * 1024, 1024)])
                    xs = stream.tile([P, KT, 1024], f16, tag="stream16",
                                     name="xs", bufs=3)
                    for half in range(2):
                        if lc == 0:
                            for q in range(2):
                                qs = ds(half * 512 + q * 256, 256)
                                nc.vector.tensor_copy(out=xs[:, :, qs],
                                                      in_=xs32[:, :, qs])
                        else:
                            hs = ds(half * 512, 512)
                            nc.vector.tensor_copy(out=xs[:, :, hs],
                                                  in_=xs32[:, :, hs])
                        for j in range(4):
                            jj = half * 4 + j
                            lt = lc * 8 + jj
                            pt = ps.tile([P, C], f16, tag="w2", bufs=2,
                                         name="pt")
                            for ci in range(KT):
                                nc.tensor.transpose(
                                    pt[:, ds(ci * P, P)],
                                    xs[:, ci, ds(jj * P, P)], ident16[:])
                            if lt % 2 == 0:
                                nc.scalar.copy(out=y_sb[:, lt, :], in_=pt[:])
                            else:
                                nc.vector.tensor_copy(out=y_sb[:, lt, :],
                                                      in_=pt[:])
                        for j in range(4):
                            lt = lc * 8 + half * 4 + j
                            for m in range(KT):
                                nc.tensor.matmul(
                                    g_ps[m][:], y_sb[:, lt, ds(m * P, P)],
                                    y_sb[:, lt, :],
                                    start=(lt == 0), stop=(lt == LT - 1))

                def finalize():
                    g_ps = state["g_ps"]
                    gh = gpool.tile([P, KT, C], f16, tag="Gh", name=f"gh{s}")
                    gl = gpool.tile([P, KT, C], f16, tag="Gl", name=f"gl{s}")
                    for m in range(KT):
                        nc.scalar.copy(out=gh[:, m, :], in_=g_ps[m][:])
                        nc.vector.tensor_sub(gl[:, m, :], g_ps[m][:],
                                             gh[:, m, :])
                    ghs[s], gls[s] = gh, gl

                return chunk, finalize

            def phase_S(s):
                """S = Wq G Wk^T in hi/lo fp16 (3 passes); softmax -> A fp16."""
                gh, gl = ghs[s], gls[s]
                hth = sbuf.tile([P, KT, C], f16, tag="HTh", name=f"hth{s}")
                htl = sbuf.tile([P, KT, C], f16, tag="HTl", name=f"htl{s}")
                e_sb = sbuf.tile([P, KT, C], f32, tag="E", name=f"e{s}")
                a16_sb = sbuf.tile([P, KT, C], f16, tag="A", name=f"a{s}")
                for m in range(KT):
                    ht_ps = ps.tile([P, C], f32, tag="w2", bufs=2, name="ht_ps")
                    for k in range(KT):
                        first, last = (k == 0), (k == KT - 1)
                        nc.tensor.matmul(ht_ps[:], gh[:, k, ds(m * P, P)],
                                         wqTh[:, k, :], start=first, stop=False)
                        nc.tensor.matmul(ht_ps[:], gh[:, k, ds(m * P, P)],
                                         wqTl[:, k, :], start=False, stop=False)
                        nc.tensor.matmul(ht_ps[:], gl[:, k, ds(m * P, P)],
                                         wqTh[:, k, :], start=False, stop=last)
                    nc.scalar.copy(out=hth[:, m, :], in_=ht_ps[:])
                    nc.vector.tensor_sub(htl[:, m, :], ht_ps[:], hth[:, m, :])
                for m in range(KT):
                    s_ps = ps.tile([P, C], f32, tag="w2", bufs=2, name="s_ps")
                    for k in range(KT):
                        first, last = (k == 0), (k == KT - 1)
                        nc.tensor.matmul(s_ps[:], hth[:, k, ds(m * P, P)],
                                         wkTh[:, k, :], start=first, stop=False)
                        nc.tensor.matmul(s_ps[:], hth[:, k, ds(m * P, P)],
                                         wkTl[:, k, :], start=False, stop=False)
                        nc.tensor.matmul(s_ps[:], htl[:, k, ds(m * P, P)],
                                         wkTh[:, k, :], start=False, stop=last)
                    negmax = stats.tile([P, 1], f32, tag="negmax", name="negmax")
                    nc.vector.reduce_max(
                        out=negmax[:], in_=s_ps[:], axis=mybir.AxisListType.X,
                        negate=True)
                    rowsum = stats.tile([P, 1], f32, tag="rowsum", name="rowsum")
                    nc.scalar.activation(
                        out=e_sb[:, m, :], in_=s_ps[:], func=AF.Exp,
                        bias=negmax[:], scale=1.0, accum_out=rowsum[:])
                    rinv = stats.tile([P, 1], f32, tag="rinv", name="rinv")
                    nc.vector.reciprocal(out=rinv[:], in_=rowsum[:])
                    nc.vector.tensor_scalar_mul(
                        a16_sb[:, m, :], e_sb[:, m, :], rinv[:])
                a16s[s] = a16_sb

            def phase_AT(s):
                a16_sb = a16s[s]
                at_sb = sbuf.tile([P, KT, C], f16, tag="AT", name=f"at{s}")
                for kt in range(KT):
                    at_ps = ps.tile([P, C], f16, tag="w2", bufs=2, name="at_ps")
                    for qi in range(KT):
                        nc.tensor.transpose(
                            at_ps[:, ds(qi * P, P)], a16_sb[:, qi, ds(kt * P, P)],
                            ident16[:])
                    nc.scalar.copy(out=at_sb[:, kt, :], in_=at_ps[:])
                ats[s] = at_sb

            def phase_MT(s):
                at_sb = ats[s]
                mt_sb = sbuf.tile([P, KT, C], f16, tag="MT", name=f"mt{s}")
                for m in range(KT):
                    mt_ps = ps.tile([P, C], f32, tag="w2", bufs=2, name="mt_ps")
                    for k in range(KT):
                        nc.tensor.matmul(
                            mt_ps[:], wv16_sb[:, k, ds(m * P, P)], at_sb[:, k, :],
                            start=(k == 0), stop=(k == KT - 1))
                    nc.scalar.activation(
                        out=mt_sb[:, m, :], in_=mt_ps[:], func=AF.Copy,
                        bias=0.0, scale=gama_sb[:])
                mts[s] = mt_sb

            def out_prefetch(s):
                """Early DMA + cast for the first OUT super-chunk of sample s."""
                x_s = x_re(s)
                xn = stream.tile([P, KT, 1024], f32, tag="stream", name="xn")
                nc.sync.dma_start(out=xn[:], in_=x_s[:, :, ds(0, 1024)])
                xn16 = stream.tile([P, KT, 1024], f16, tag="stream16",
                                   name="xn16", bufs=3)
                for half in range(2):
                    hs = ds(half * 512, 512)
                    nc.scalar.copy(out=xn16[:, :, hs], in_=xn[:, :, hs])
                pre[s] = (xn, xn16)

            def out_phase(s):
                """Returns chunk_fns for the OUT phase of sample s."""
                x_s = x_re(s)
                y_s = y_d.ap()[s].rearrange("(t p) l -> p t l", p=P)

                def chunk(ncx):
                    mt_sb = mts[s]
                    if ncx == 0 and s in pre:
                        xn, xn16 = pre.pop(s)
                    else:
                        xn = stream.tile([P, KT, 1024], f32, tag="stream",
                                         name="xn")
                        nc.sync.dma_start(out=xn[:],
                                          in_=x_s[:, :, ds(ncx * 1024, 1024)])
                        xn16 = stream.tile([P, KT, 1024], f16, tag="stream16",
                                           name="xn16", bufs=3)
                    stg = stage.tile([P, KT, 1024], f32, tag="stage", name="stg")
                    for half in range(2):
                        hs = ds(half * 512, 512)
                        if ncx != 0:
                            nc.scalar.copy(out=xn16[:, :, hs], in_=xn[:, :, hs])
                        o_ps = [ps.tile([P, 512], f32, tag="oacc", bufs=2,
                                        name=f"o{m}") for m in range(KT)]
                        for m in range(KT):
                            for k in range(KT):
                                nc.tensor.matmul(
                                    o_ps[m][:], mt_sb[:, k, ds(m * P, P)],
                                    xn16[:, k, hs],
                                    start=(k == 0), stop=(k == KT - 1))
                            nc.vector.tensor_add(stg[:, m, hs], o_ps[m][:],
                                                 xn[:, m, hs])
                    if ncx == NCH2 - 1:
                        for half in range(2):
                            nc.sync.dma_start(
                                out=y_s[:, :, ds(ncx * 1024 + half * 512, 512)],
                                in_=stg[:, :, ds(half * 512, 512)])
                    else:
                        nc.sync.dma_start(out=y_s[:, :, ds(ncx * 1024, 1024)],
                                          in_=stg[:])

                return chunk

            # ---------------- schedule ----------------
            for _rep in range(repeat):
                tg0_chunk, tg0_fin = tg_phase(0)
                tg0_chunk(0)
                tg0_chunk(1)
                if _rep == 0:
                    phase_W()
                tg0_chunk(2)
                tg0_chunk(3)
                tg0_fin()
                tg1_chunk, tg1_fin = tg_phase(1)
                tg1_chunk(0)
                phase_S(0)
                phase_AT(0)
                phase_MT(0)
                out_prefetch(0)
                out0_chunk = out_phase(0)
                out0_chunk(0)
                tg1_chunk(1)
                out0_chunk(1)
                tg1_chunk(2)
                out0_chunk(2)
                tg1_chunk(3)
                tg1_fin()
                out0_chunk(3)
                out_prefetch(1)
                phase_S(1)
                phase_AT(1)
                phase_MT(1)
                out1_chunk = out_phase(1)
                for i in range(NCH2):
                    out1_chunk(i)

    nc.finalize()
    return nc


_NC_CACHE = {}


def _get_nc():
    if "nc" not in _NC_CACHE:
        _NC_CACHE["nc"] = _build()
    return _NC_CACHE["nc"]


def _run(inputs, trace=False):
    from concourse.bass_utils import run_bass_kernel_spmd

    x = np.ascontiguousarray(np.asarray(inputs["x"], dtype=np.float32)
                             .reshape(N, C, L))
    wq = np.ascontiguousarray(np.asarray(inputs["Wq"], dtype=np.float32))
    wk = np.ascontiguousarray(np.asarray(inputs["Wk"], dtype=np.float32))
    wv = np.ascontiguousarray(np.asarray(inputs["Wv"], dtype=np.float32))
    gama = np.ascontiguousarray(np.asarray(inputs["gama"], dtype=np.float32)
                                .reshape(1))

    nc = _get_nc()
    ident16 = np.eye(P, dtype=np.float16)
    ident32 = np.eye(P, dtype=np.float32)
    in_maps = [
        {"x": x[c * NS:(c + 1) * NS], "Wq": wq, "Wk": wk, "Wv": wv,
         "gama": gama, "ident16_in": ident16, "ident32_in": ident32}
        for c in range(N_CORES)
    ]
    res = run_bass_kernel_spmd(nc, in_maps, core_ids=list(range(N_CORES)),
                               trace=trace)
    y = np.concatenate([r["y"][None] for r in res.results], axis=0)
    y = y.reshape(N, C, H, W).astype(np.float32)
    return y, res


def kernel(**inputs):
    y, _ = _run(inputs, trace=False)
    return y



# revision 6
# speedup vs baseline: 1.1076x; 1.1076x over previous
"""ChannelAttention TRN2 Bass kernel.

Math (per sample):
  xf = x.reshape(C, L)
  G  = xf @ xf.T                      [C, C]   (Gram over spatial dim; symmetric)
  S  = Wq @ G @ Wk.T                  [C, C]   (== Q @ K.T)
  A  = softmax(S, axis=1)
  M  = A @ Wv                         [C, C]
  y  = gama * (M @ xf) + xf           [C, L]

Sharding: data-parallel over batch (16 samples / 8 cores = 2 per core).

Precision:
  - x cast to fp16 once (resident in SBUF); Gram + S-chain in plain fp16
    (fp32 PSUM accum).  Gram exploits symmetry: only upper-triangle tiles
    computed, lower filled by PE transpose.
  - Value path in fp8 e4m3 with DoubleRow (2 c-tiles packed per PE cell):
    a8 = fp8(A), wv8 = fp8(Wv), mt8 = fp8(32*gama*(A@Wv)^T), out matmul
    o = mt8 @ fp8(x).  Residual y = o/32 + x(fp16) fused on GpSimd via
    scalar_tensor_tensor.
  - Softmax in fp32.

Schedule: phases of the two samples interleaved so TG1's DMA-in overlaps
S0/OUT0 compute and OUT0's DMA-out overlaps TG1 compute.
"""
import numpy as np

N_CORES = 8
N, C, H, W = 16, 512, 64, 64
L = H * W            # 4096
NS = N // N_CORES    # samples per core
P = 128              # partitions
KT = C // P          # 4 c-tiles
LT = L // P          # 32 l-tiles
NCH2 = L // 1024     # 4 super-chunks of 1024 columns
RS = 32.0            # fp8 value-path scale: mt8 = RS*gama*(A@Wv)^T


def _build(trace_scopes=False, repeat=None):
    import os
    import concourse.bass as bass
    import concourse.mybir as mybir
    import concourse.tile as tile
    from concourse import bacc
    from concourse.bass import ds

    f32 = mybir.dt.float32
    f16 = mybir.dt.float16
    f8 = mybir.dt.float8e4
    AF = mybir.ActivationFunctionType
    ALU = mybir.AluOpType
    DR = mybir.MatmulPerfMode.DoubleRow

    if repeat is None:
        repeat = int(os.environ.get("KERNEL_BUILD_REPEAT", "1"))

    nc = bacc.Bacc("TRN2", debug=False)
    x_d = nc.dram_tensor("x", [NS, C, L], f32, kind="ExternalInput")
    wq_d = nc.dram_tensor("Wq", [C, C], f32, kind="ExternalInput")
    wk_d = nc.dram_tensor("Wk", [C, C], f32, kind="ExternalInput")
    wv_d = nc.dram_tensor("Wv", [C, C], f32, kind="ExternalInput")
    gama_d = nc.dram_tensor("gama", [1], f32, kind="ExternalInput")
    i16_d = nc.dram_tensor("ident16_in", [P, P], f16, kind="ExternalInput")
    i32_d = nc.dram_tensor("ident32_in", [P, P], f32, kind="ExternalInput")
    y_d = nc.dram_tensor("y", [NS, C, L], f32, kind="ExternalOutput")

    with tile.TileContext(nc) as tc:
        from contextlib import ExitStack
        ctx = ExitStack()
        with ctx:
            consts = ctx.enter_context(tc.tile_pool(name="consts", bufs=1))
            wpool = ctx.enter_context(tc.tile_pool(name="wpool", bufs=1))
            xpool = ctx.enter_context(tc.tile_pool(name="xpool", bufs=1))
            sbuf = ctx.enter_context(tc.tile_pool(name="sbuf", bufs=1))
            gpool = ctx.enter_context(tc.tile_pool(name="gpool", bufs=1))
            stream = ctx.enter_context(tc.tile_pool(name="stream", bufs=2))
            stage = ctx.enter_context(tc.tile_pool(name="stage", bufs=2))
            stats = ctx.enter_context(tc.tile_pool(name="stats", bufs=4))
            ps = ctx.enter_context(tc.tile_pool(name="ps", bufs=1, space="PSUM"))

            ident16 = consts.tile([P, P], f16)
            nc.sync.dma_start(out=ident16[:], in_=i16_d.ap())
            ident = consts.tile([P, P], f32)
            nc.sync.dma_start(out=ident[:], in_=i32_d.ap())
            ident8 = consts.tile([P, P], f8)
            nc.vector.tensor_copy(out=ident8[:], in_=ident16[:])
            gama_sb = consts.tile([P, 1], f32)
            nc.gpsimd.dma_start(out=gama_sb[:], in_=gama_d.ap().to_broadcast((P, 1)))
            # rs_gama = RS*gama, per-partition scalar for the mt8 cast
            rs_gama = consts.tile([P, 1], f32)
            nc.vector.tensor_scalar_mul(rs_gama[:], gama_sb[:], RS)

            # resident fp16 copy of x for both samples (Gram source + residual)
            xs16 = [xpool.tile([P, KT, L], f16, name=f"xs16_{s}")
                    for s in range(NS)]

            # weight tiles (filled by phase_W)
            wv8_sb = wpool.tile([P, KT, C], f8)    # Wv natural: [k part, k-tile, c]
            wqT = wpool.tile([P, KT, C], f16)      # Wq^T: [c part, c-tile, q]
            wkT = wpool.tile([P, KT, C], f16)

            def phase_W():
                wvn = stream.tile([P, KT, C], f32, tag="wstream", bufs=2,
                                  name="wvn")
                nc.sync.dma_start(
                    out=wvn[:], in_=wv_d.ap().rearrange("(t p) c -> p t c", p=P))
                nc.vector.tensor_copy(out=wv8_sb[:], in_=wvn[:])
                for w_d, wT in ((wq_d, wqT), (wk_d, wkT)):
                    wn = stream.tile([P, KT, C], f32, tag="wstream", bufs=2,
                                     name="wn")
                    nc.sync.dma_start(
                        out=wn[:], in_=w_d.ap().rearrange("(t p) c -> p t c", p=P))
                    for ct in range(KT):
                        ptw = ps.tile([P, C], f32, tag="w2", bufs=2, name="ptw")
                        for qt in range(KT):
                            nc.tensor.transpose(
                                ptw[:, ds(qt * P, P)], wn[:, qt, ds(ct * P, P)],
                                ident[:])
                        nc.scalar.copy(out=wT[:, ct, :], in_=ptw[:])

            # per-sample state kept across interleaved phases
            ghs = [None] * NS
            a8s = [None] * NS
            at8s = [None] * NS
            mt8s = [None] * NS

            def x_re(s):
                return x_d.ap()[s].rearrange("(t p) l -> p t l", p=P)  # [128,KT,L]

            def tg_phase(s):
                """Returns (chunk_fns, finalize_fn): stream x 2MB super-chunks,
                cast fp16 into resident xs16, transpose on PE, accumulate the
                upper-triangle Gram tiles per chunk."""
                x_s = x_re(s)
                state = {}

                def chunk(lc):
                    if lc == 0:
                        # g_ps[m] = G[m-tile, m*128:512], width 512-128*m
                        state["g_ps"] = [
                            ps.tile([P, C - m * P], f32, tag=f"acc{m}", bufs=1,
                                    name=f"g{s}_{m}") for m in range(KT)]
                    g_ps = state["g_ps"]
                    xs32 = stream.tile([P, KT, 1024], f32, tag="stream",
                                       name="xs32")
                    if lc == 0:
                        for q in range(4):
                            nc.sync.dma_start(
                                out=xs32[:, :, ds(q * 256, 256)],
                                in_=x_s[:, :, ds(q * 256, 256)])
                    else:
                        nc.sync.dma_start(out=xs32[:],
                                          in_=x_s[:, :, ds(lc * 1024, 1024)])
                    for half in range(2):
                        cs = ds(lc * 1024 + half * 512, 512)
                        if lc == 0:
                            for q in range(2):
                                qs = ds(half * 512 + q * 256, 256)
                                cq = ds(lc * 1024 + half * 512 + q * 256, 256)
                                nc.vector.tensor_copy(out=xs16[s][:, :, cq],
                                                      in_=xs32[:, :, qs])
                        else:
                            hs = ds(half * 512, 512)
                            nc.vector.tensor_copy(out=xs16[s][:, :, cs],
                                                  in_=xs32[:, :, hs])
                        for j in range(4):
                            jj = half * 4 + j
                            lt = lc * 8 + jj
                            pt = ps.tile([P, C], f16, tag="w2", bufs=2,
                                         name="pt")
                            for ci in range(KT):
                                nc.tensor.transpose(
                                    pt[:, ds(ci * P, P)],
                                    xs16[s][:, ci, ds(lt * P, P)], ident16[:])
                            yt = sbuf.tile([P, C], f16, tag="yt", bufs=6,
                                           name="yt")
                            if lt % 2 == 0:
                                nc.scalar.copy(out=yt[:], in_=pt[:])
                            else:
                                nc.vector.tensor_copy(out=yt[:], in_=pt[:])
                            for m in range(KT):
                                nc.tensor.matmul(
                                    g_ps[m][:], yt[:, ds(m * P, P)],
                                    yt[:, ds(m * P, C - m * P)],
                                    start=(lt == 0), stop=(lt == LT - 1))

                def finalize():
                    g_ps = state["g_ps"]
                    gh = gpool.tile([P, KT, C], f16, tag="Gh", bufs=2,
                                    name=f"gh{s}")
                    for m in range(KT):
                        nc.scalar.copy(out=gh[:, m, ds(m * P, C - m * P)],
                                       in_=g_ps[m][:])
                    # fill lower tiles (r > c): gh[r, c] = gh[c, r]^T
                    for r in range(1, KT):
                        ptl = ps.tile([P, KT * P], f16, tag="w2", bufs=2,
                                      name="ptl")
                        for c in range(r):
                            nc.tensor.transpose(
                                ptl[:, ds(c * P, P)],
                                gh[:, c, ds(r * P, P)], ident16[:])
                        nc.vector.tensor_copy(out=gh[:, r, ds(0, r * P)],
                                              in_=ptl[:, ds(0, r * P)])
                    ghs[s] = gh

                return chunk, finalize

            def phase_S(s):
                """S = Wq G Wk^T in fp16; softmax -> A fp8."""
                gh = ghs[s]
                hth = sbuf.tile([P, KT, C], f16, tag="HTh", name=f"hth{s}")
                e_sb = sbuf.tile([P, KT, C], f32, tag="E", name=f"e{s}")
                a16_sb = sbuf.tile([P, KT, C], f16, tag="A", bufs=2,
                                   name=f"a{s}")
                for m in range(KT):
                    ht_ps = ps.tile([P, C], f32, tag="w2", bufs=2, name="ht_ps")
                    for k in range(KT):
                        nc.tensor.matmul(ht_ps[:], gh[:, k, ds(m * P, P)],
                                         wqT[:, k, :],
                                         start=(k == 0), stop=(k == KT - 1))
                    nc.scalar.copy(out=hth[:, m, :], in_=ht_ps[:])
                for m in range(KT):
                    s_ps = ps.tile([P, C], f32, tag="w2", bufs=2, name="s_ps")
                    for k in range(KT):
                        nc.tensor.matmul(s_ps[:], hth[:, k, ds(m * P, P)],
                                         wkT[:, k, :],
                                         start=(k == 0), stop=(k == KT - 1))
                    negmax = stats.tile([P, 1], f32, tag="negmax", name="negmax")
                    nc.vector.reduce_max(
                        out=negmax[:], in_=s_ps[:], axis=mybir.AxisListType.X,
                        negate=True)
                    rowsum = stats.tile([P, 1], f32, tag="rowsum", name="rowsum")
                    nc.scalar.activation(
                        out=e_sb[:, m, :], in_=s_ps[:], func=AF.Exp,
                        bias=negmax[:], scale=1.0, accum_out=rowsum[:])
                    rinv = stats.tile([P, 1], f32, tag="rinv", name="rinv")
                    nc.vector.reciprocal(out=rinv[:], in_=rowsum[:])
                    nc.vector.tensor_scalar_mul(
                        a16_sb[:, m, :], e_sb[:, m, :], rinv[:])
                a8s[s] = a16_sb

            def phase_AT(s):
                a16_sb = a8s[s]
                at8_sb = sbuf.tile([P, KT, C], f8, tag="AT", bufs=2,
                                   name=f"at{s}")
                for kt in range(KT):
                    at_ps = ps.tile([P, C], f16, tag="w2", bufs=2, name="at_ps")
                    for qi in range(KT):
                        nc.tensor.transpose(
                            at_ps[:, ds(qi * P, P)], a16_sb[:, qi, ds(kt * P, P)],
                            ident16[:])
                    nc.scalar.copy(out=at8_sb[:, kt, :], in_=at_ps[:])
                at8s[s] = at8_sb

            def phase_MT(s):
                """mt8 = RS*gama*(A @ Wv)^T in fp8 via DoubleRow."""
                at8_sb = at8s[s]
                mt8_sb = sbuf.tile([P, KT, C], f8, tag="MT", bufs=2,
                                   name=f"mt{s}")
                for m in range(KT):
                    mt_ps = ps.tile([P, C], f32, tag="w2", bufs=2, name="mt_ps")
                    for kp in range(KT // 2):
                        kslice = slice(2 * kp, 2 * kp + 2)
                        nc.tensor.matmul(
                            mt_ps[:], wv8_sb[:, kslice, ds(m * P, P)],
                            at8_sb[:, kslice, :],
                            start=(kp == 0), stop=(kp == KT // 2 - 1),
                            perf_mode=DR)
                    nc.scalar.activation(
                        out=mt8_sb[:, m, :], in_=mt_ps[:], func=AF.Copy,
                        bias=0.0, scale=rs_gama[:])
                mt8s[s] = mt8_sb

            def out_phase(s):
                """Returns chunk_fns for the OUT phase of sample s:
                o = mt8 @ fp8(x) via DoubleRow; y = o/RS + x(fp16)."""
                y_s = y_d.ap()[s].rearrange("(t p) l -> p t l", p=P)

                def chunk(ncx):
                    mt_sb = mt8s[s]
                    xn8 = stream.tile([P, KT, 1024], f8, tag="stream8",
                                      name="xn8", bufs=2)
                    stg = stage.tile([P, KT, 1024], f32, tag="stage", name="stg")
                    for half in range(2):
                        hs = ds(half * 512, 512)
                        cs = ds(ncx * 1024 + half * 512, 512)
                        nc.gpsimd.tensor_copy(out=xn8[:, :, hs],
                                              in_=xs16[s][:, :, cs])
                        o_ps = [ps.tile([P, 512], f32, tag="oacc", bufs=2,
                                        name=f"o{m}") for m in range(KT)]
                        for m in range(KT):
                            for kp in range(KT // 2):
                                kslice = slice(2 * kp, 2 * kp + 2)
                                nc.tensor.matmul(
                                    o_ps[m][:], mt_sb[:, kslice, ds(m * P, P)],
                                    xn8[:, kslice, hs],
                                    start=(kp == 0), stop=(kp == KT // 2 - 1),
                                    perf_mode=DR)
                            nc.vector.scalar_tensor_tensor(
                                out=stg[:, m, hs], in0=o_ps[m][:],
                                scalar=1.0 / RS, in1=xs16[s][:, m, cs],
                                op0=ALU.mult, op1=ALU.add)
                    if ncx == NCH2 - 1:
                        for half in range(2):
                            nc.sync.dma_start(
                                out=y_s[:, :, ds(ncx * 1024 + half * 512, 512)],
                                in_=stg[:, :, ds(half * 512, 512)])
                    else:
                        nc.sync.dma_start(out=y_s[:, :, ds(ncx * 1024, 1024)],
                                          in_=stg[:])

                return chunk

            # ---------------- schedule ----------------
            for _rep in range(repeat):
                tg0_chunk, tg0_fin = tg_phase(0)
                tg0_chunk(0)
                tg0_chunk(1)
                if _rep == 0:
                    phase_W()
                tg0_chunk(2)
                tg0_chunk(3)
                tg0_fin()
                tg1_chunk, tg1_fin = tg_phase(1)
                tg1_chunk(0)
                phase_S(0)
                phase_AT(0)
                phase_MT(0)
                out0_chunk = out_phase(0)
                out0_chunk(0)
                tg1_chunk(1)
                out0_chunk(1)
                tg1_chunk(2)
                out0_chunk(2)
                tg1_chunk(3)
                tg1_fin()
                out0_chunk(3)
                phase_S(1)
                phase_AT(1)
                phase_MT(1)
                out1_chunk = out_phase(1)
                for i in range(NCH2):
                    out1_chunk(i)

    nc.finalize()
    return nc


_NC_CACHE = {}


def _get_nc():
    if "nc" not in _NC_CACHE:
        _NC_CACHE["nc"] = _build()
    return _NC_CACHE["nc"]


def _run(inputs, trace=False):
    from concourse.bass_utils import run_bass_kernel_spmd

    x = np.ascontiguousarray(np.asarray(inputs["x"], dtype=np.float32)
                             .reshape(N, C, L))
    wq = np.ascontiguousarray(np.asarray(inputs["Wq"], dtype=np.float32))
    wk = np.ascontiguousarray(np.asarray(inputs["Wk"], dtype=np.float32))
    wv = np.ascontiguousarray(np.asarray(inputs["Wv"], dtype=np.float32))
    gama = np.ascontiguousarray(np.asarray(inputs["gama"], dtype=np.float32)
                                .reshape(1))

    nc = _get_nc()
    ident16 = np.eye(P, dtype=np.float16)
    ident32 = np.eye(P, dtype=np.float32)
    in_maps = [
        {"x": x[c * NS:(c + 1) * NS], "Wq": wq, "Wk": wk, "Wv": wv,
         "gama": gama, "ident16_in": ident16, "ident32_in": ident32}
        for c in range(N_CORES)
    ]
    res = run_bass_kernel_spmd(nc, in_maps, core_ids=list(range(N_CORES)),
                               trace=trace)
    y = np.concatenate([r["y"][None] for r in res.results], axis=0)
    y = y.reshape(N, C, H, W).astype(np.float32)
    return y, res


def kernel(**inputs):
    y, _ = _run(inputs, trace=False)
    return y
